# revision 1
# baseline (speedup 1.0000x reference)
"""Trainium2 Bass kernel for nn_CNNT_enhanced_denoising_runtime_53704271069472.

Computes, distributed across 8 NeuronCores:
    q/k/v = conv3x3(x, w?, b?)          (image-sharded: B*T=128 imgs, 16/core)
    att   = causal-softmax(q @ k^T / sqrt(D)) per (batch, head)
    y     = att @ v                      (head-sharded: 16 (b,head) pairs, 2/core)
    out   = conv3x3(y, wo, bo)           (image-sharded)

Three SPMD launches with host-side resharding between them. Convs are done as
matmuls over K = (3 kx-taps x 16 ch [+ ones bias row]) x 2 ky-rows = 97(+48)
against kx-pre-shifted zero-padded image planes built on the host; the 3x3
kernel's third ky row is a second accumulating matmul with an AP row offset.
Compute dtype bf16 (fp32 PSUM accumulation).
"""
import sys
import numpy as np

sys.path.insert(0, "/opt/trn_rl_repo")

import ml_dtypes  # noqa: E402
import concourse.bacc as bacc  # noqa: E402
import concourse.tile as tile  # noqa: E402
import concourse.bass as bass  # noqa: E402
from concourse import mybir, bass_utils  # noqa: E402

BF16 = mybir.dt.bfloat16
F32 = mybir.dt.float32
NPBF16 = ml_dtypes.bfloat16

B, T, C, H, W, O = 2, 64, 16, 128, 128, 16
HP, WP = H + 2, W + 2
HW = H * W
NH, HC = 8, 2
D = HC * HW
SCALE = float(1.0 / np.sqrt(np.float32(D)))
NCORES = 8
IMGS = B * T
IPC = IMGS // NCORES  # images per core
NPL = 98  # plane rows: 48 (ky0 kx-taps) + ones + 48 (ky1) + pad

_BUILD_CACHE = {}


# ---------------- device programs ----------------

def _build_l1():
    nc = bacc.Bacc("TRN2", target_bir_lowering=False, debug=False)
    planes = nc.dram_tensor("planes", (IPC, NPL, HP * WP), BF16, kind="ExternalInput")
    lhsT0 = nc.dram_tensor("lhsT0", (97, 48), BF16, kind="ExternalInput")
    lhsT1 = nc.dram_tensor("lhsT1", (48, 48), BF16, kind="ExternalInput")
    qkv = nc.dram_tensor("qkv_out", (IPC, 128, 8192), BF16, kind="ExternalOutput")

    with tile.TileContext(nc) as tc:
        with tc.tile_pool(name="w", bufs=1) as wpool, \
             tc.tile_pool(name="pl", bufs=2) as plpool, \
             tc.tile_pool(name="st", bufs=2) as stpool, \
             tc.tile_pool(name="ps", bufs=4, space="PSUM") as pspool:
            w0 = wpool.tile([97, 48], BF16, tag="w0")
            w1 = wpool.tile([48, 48], BF16, tag="w1")
            nc.sync.dma_start(w0[:], lhsT0.ap())
            nc.sync.dma_start(w1[:], lhsT1.ap())

            def rhs_view(pt, nrows, blk, ky):
                base = (blk * 4 + ky) * WP
                return pt[0:nrows, base:base + 4 * WP].rearrange(
                    "p (h w) -> p h w", w=WP)[:, :, 0:W]

            for img in range(IPC):
                pt = plpool.tile([NPL, HP * WP], BF16)
                nc.sync.dma_start(pt[:], planes.ap()[img])
                stage = stpool.tile([128, 8192], BF16)
                for blk2 in range(16):
                    ps = pspool.tile([128, 512], F32)
                    for half in range(2):
                        blk = blk2 * 2 + half
                        nc.tensor.matmul(ps[half * 64:half * 64 + 48, :],
                                         w0[:], rhs_view(pt, 97, blk, 0),
                                         start=True, stop=False,
                                         tile_position=(0, half * 64))
                        nc.tensor.matmul(ps[half * 64:half * 64 + 48, :],
                                         w1[:], rhs_view(pt, 48, blk, 2),
                                         start=False, stop=True,
                                         tile_position=(0, half * 64))
                    nc.vector.tensor_copy(stage[:, blk2 * 512:(blk2 + 1) * 512], ps[:])
                nc.sync.dma_start(qkv.ap()[img], stage[:])
    nc.compile()
    return nc


def _build_l2():
    nc = bacc.Bacc("TRN2", target_bir_lowering=False, debug=False)
    qs = nc.dram_tensor("qs", (4, T, HW), BF16, kind="ExternalInput")
    ks = nc.dram_tensor("ks", (4, T, HW), BF16, kind="ExternalInput")
    vs = nc.dram_tensor("vs", (4, T, HW), BF16, kind="ExternalInput")
    mask = nc.dram_tensor("mask", (T, T), F32, kind="ExternalInput")
    ident = nc.dram_tensor("ident", (T, T), BF16, kind="ExternalInput")
    ys = nc.dram_tensor("ys", (2, 128, HW), BF16, kind="ExternalOutput")

    with tile.TileContext(nc) as tc:
        with tc.tile_pool(name="cst", bufs=1) as cst, \
             tc.tile_pool(name="qk", bufs=3) as qkpool, \
             tc.tile_pool(name="sm", bufs=2) as smpool, \
             tc.tile_pool(name="v", bufs=3) as vpool, \
             tc.tile_pool(name="yst", bufs=2) as ypool, \
             tc.tile_pool(name="pst", bufs=1, space="PSUM") as pstpool, \
             tc.tile_pool(name="psy", bufs=3, space="PSUM") as psypool, \
             tc.tile_pool(name="psl", bufs=1, space="PSUM") as pslpool:
            mask_t = cst.tile([T, T], F32, tag="mask")
            nc.sync.dma_start(mask_t[:], mask.ap())
            id_t = cst.tile([T, T], BF16, tag="ident")
            nc.sync.dma_start(id_t[:], ident.ap())

            lg_ps = [pslpool.tile([128, 128], F32, tag=f"lg{h}", name=f"lg{h}")
                     for h in range(2)]
            for blk in range(128):
                qt = qkpool.tile([128, 256], BF16, tag="qT")
                kt = qkpool.tile([128, 256], BF16, tag="kT")
                src_q = qs.ap()[:, :, blk * 128:(blk + 1) * 128].rearrange("c t p -> (c t) p")
                src_k = ks.ap()[:, :, blk * 128:(blk + 1) * 128].rearrange("c t p -> (c t) p")
                nc.sync.dma_start_transpose(qt[:], src_q)
                nc.sync.dma_start_transpose(kt[:], src_k)
                for h in range(2):
                    nc.tensor.matmul(lg_ps[h][:],
                                     qt[:, h * 128:(h + 1) * 128],
                                     kt[:, h * 128:(h + 1) * 128],
                                     start=(blk == 0), stop=(blk == 127))

            attTs = []
            for h in range(2):
                lg = smpool.tile([T, T], F32, tag="lg")
                nc.vector.tensor_copy(lg[:], lg_ps[h][0:64, 0:64])
                nc.vector.tensor_add(lg[:], lg[:], lg_ps[h][64:128, 64:128])
                nc.vector.tensor_scalar(lg[:], lg[:], SCALE, None,
                                        op0=mybir.AluOpType.mult)
                nc.vector.tensor_add(lg[:], lg[:], mask_t[:])
                mx = smpool.tile([T, 1], F32, tag="mx")
                nc.vector.reduce_max(mx[:], lg[:], axis=mybir.AxisListType.X, negate=True)
                e = smpool.tile([T, T], F32, tag="e")
                sm_acc = smpool.tile([T, 1], F32, tag="smacc")
                nc.scalar.activation(e[:], lg[:], mybir.ActivationFunctionType.Exp,
                                     bias=mx[:], scale=1.0, accum_out=sm_acc[:])
                rc = smpool.tile([T, 1], F32, tag="rc")
                nc.vector.reciprocal(rc[:], sm_acc[:])
                att = smpool.tile([T, T], BF16, tag="att")
                nc.vector.tensor_scalar(att[:], e[:], rc[:], None,
                                        op0=mybir.AluOpType.mult)
                ps_t = pstpool.tile([T, T], BF16, tag="pst")
                nc.tensor.transpose(ps_t[:], att[:], id_t[:])
                attT = smpool.tile([T, T], BF16, tag=f"attT{h}", name=f"attT{h}")
                nc.vector.tensor_copy(attT[:], ps_t[:])
                attTs.append(attT)

            for p in range(2):
                yst = ypool.tile([128, HW], BF16, tag="yst")
                for ci in range(2):
                    ch = 2 * p + ci
                    attT = attTs[ch // 2]
                    for vb in range(4):
                        vt = vpool.tile([T, 4096], BF16, tag="vt")
                        nc.sync.dma_start(vt[:], vs.ap()[ch, :, vb * 4096:(vb + 1) * 4096])
                        for j in range(8):
                            ps_y = psypool.tile([T, 512], F32, tag="psy")
                            nc.tensor.matmul(ps_y[:], attT[:],
                                             vt[:, j * 512:(j + 1) * 512],
                                             start=True, stop=True)
                            col = vb * 4096 + j * 512
                            nc.vector.tensor_copy(
                                yst[ci * 64:ci * 64 + 64, col:col + 512], ps_y[:])
                nc.sync.dma_start(ys.ap()[p], yst[:])
    nc.compile()
    return nc


def _build_l3():
    nc = bacc.Bacc("TRN2", target_bir_lowering=False, debug=False)
    planes = nc.dram_tensor("planes", (IPC, NPL, HP * WP), BF16, kind="ExternalInput")
    lhsT0 = nc.dram_tensor("lhsT0", (97, 16), BF16, kind="ExternalInput")
    lhsT1 = nc.dram_tensor("lhsT1", (48, 16), BF16, kind="ExternalInput")
    out = nc.dram_tensor("out", (IPC, 2, 16, 8192), F32, kind="ExternalOutput")

    with tile.TileContext(nc) as tc:
        with tc.tile_pool(name="w", bufs=1) as wpool, \
             tc.tile_pool(name="pl", bufs=2) as plpool, \
             tc.tile_pool(name="st", bufs=2) as stpool, \
             tc.tile_pool(name="ps", bufs=4, space="PSUM") as pspool:
            w0 = wpool.tile([97, 16], BF16, tag="w0")
            w1 = wpool.tile([48, 16], BF16, tag="w1")
            nc.sync.dma_start(w0[:], lhsT0.ap())
            nc.sync.dma_start(w1[:], lhsT1.ap())

            def rhs_view(pt, nrows, blk, ky):
                base = (blk * 4 + ky) * WP
                return pt[0:nrows, base:base + 4 * WP].rearrange(
                    "p (h w) -> p h w", w=WP)[:, :, 0:W]

            for img in range(IPC):
                pt = plpool.tile([NPL, HP * WP], BF16)
                nc.sync.dma_start(pt[:], planes.ap()[img])
                stage = stpool.tile([128, 8192], F32)
                for blk2 in range(16):
                    ps = pspool.tile([128, 512], F32)
                    for half in range(2):
                        blk = blk2 * 2 + half
                        po = half * 64
                        nc.tensor.matmul(ps[po:po + 16, :], w0[:],
                                         rhs_view(pt, 97, blk, 0),
                                         start=True, stop=False,
                                         tile_position=(0, po))
                        nc.tensor.matmul(ps[po:po + 16, :], w1[:],
                                         rhs_view(pt, 48, blk, 2),
                                         start=False, stop=True,
                                         tile_position=(0, po))
                    for half in range(2):
                        nc.vector.tensor_copy(
                            stage[half * 64:half * 64 + 16,
                                  blk2 * 512:blk2 * 512 + 512],
                            ps[half * 64:half * 64 + 16, :])
                nc.sync.dma_start(out.ap()[img, 0], stage[0:16, :])
                nc.sync.dma_start(out.ap()[img, 1], stage[64:80, :])
    nc.compile()
    return nc


def _get(name):
    if name not in _BUILD_CACHE:
        _BUILD_CACHE[name] = {"l1": _build_l1, "l2": _build_l2, "l3": _build_l3}[name]()
    return _BUILD_CACHE[name]


# ---------------- host-side packing ----------------

def _build_planes(imgs_chw):
    """imgs_chw: [N, 16, H, W] float32-like -> [N, 98, HP*WP] bf16."""
    N = imgs_chw.shape[0]
    xpad = np.zeros((N, C, HP, WP), np.float32)
    xpad[:, :, 1:H + 1, 1:W + 1] = imgs_chw.astype(np.float32)
    flat = xpad.reshape(N, C, HP * WP)
    p = np.zeros((N, NPL, HP * WP), np.float32)
    p[:, 0:16] = flat
    p[:, 16:32, :-1] = flat[:, :, 1:]
    p[:, 32:48, :-2] = flat[:, :, 2:]
    p[:, 48] = 1.0
    p[:, 49:97, :-WP] = p[:, 0:48, WP:]
    return p.astype(NPBF16)


def _build_lhsT(ws, bs):
    """ws: list of [O,C,3,3]; bs: list of [O] -> lhsT0 [97, 16*len], lhsT1 [48, 16*len]."""
    n = len(ws)
    m = np.zeros((3, 49, 16 * n), np.float32)
    for j, (w, b) in enumerate(zip(ws, bs)):
        for ky in range(3):
            for kx in range(3):
                m[ky, kx * 16:(kx + 1) * 16, j * 16:(j + 1) * 16] = w[:, :, ky, kx].T
        m[1, 48, j * 16:(j + 1) * 16] = b
    l0 = np.zeros((97, 16 * n), np.float32)
    l0[0:48] = m[0][0:48]
    l0[48] = m[1][48]
    l0[49:97] = m[1][0:48]
    return l0.astype(NPBF16), m[2][0:48].astype(NPBF16)


def _unpack_qkv(qkv_out):
    """[N,128,8192] bf16 -> q,k,v each [N,16,HW]."""
    N = qkv_out.shape[0]
    s = qkv_out.reshape(N, 128, 16, 512)
    out = np.empty((N, 48, 32, 512), qkv_out.dtype)
    out[:, :, 0::2] = s[:, 0:48]
    out[:, :, 1::2] = s[:, 64:112]
    out = out.reshape(N, 48, HW)
    return out[:, 0:16], out[:, 16:32], out[:, 32:48]


def _unpack_l3(o):
    """[N,2,16,8192] f32 -> [N,16,HW]; blk = blk2*2+half at [half,c,blk2*512:+512]."""
    N = o.shape[0]
    s = o.reshape(N, 2, 16, 16, 512)
    s = s.transpose(0, 2, 3, 1, 4)
    return np.ascontiguousarray(s).reshape(N, 16, HW)


# ---------------- top level ----------------

def kernel(x, wq, bq, wk, bk, wv, bv, wo, bo):
    x, wq, bq, wk, bk, wv, bv, wo, bo = (
        np.asarray(a, np.float32) for a in (x, wq, bq, wk, bk, wv, bv, wo, bo))
    ximg = x.reshape(IMGS, C, H, W)
    cores = list(range(NCORES))

    # ---- L1: q/k/v convs, image-sharded
    l0, l1 = _build_lhsT([wq, wk, wv], [bq, bk, bv])
    in_maps = [{"planes": _build_planes(ximg[c * IPC:(c + 1) * IPC]),
                "lhsT0": l0, "lhsT1": l1} for c in cores]
    res1 = bass_utils.run_bass_kernel_spmd(_get("l1"), in_maps, core_ids=cores)

    # assemble channel-major [B, 16, T, HW] bf16
    q_all = np.empty((B, 16, T, HW), NPBF16)
    k_all = np.empty_like(q_all)
    v_all = np.empty_like(q_all)
    for c in cores:
        q, k, v = _unpack_qkv(res1.results[c]["qkv_out"])
        b0 = (c * IPC) // T
        t0 = (c * IPC) % T
        q_all[b0, :, t0:t0 + IPC] = q.transpose(1, 0, 2)
        k_all[b0, :, t0:t0 + IPC] = k.transpose(1, 0, 2)
        v_all[b0, :, t0:t0 + IPC] = v.transpose(1, 0, 2)

    # ---- L2: attention, head-sharded (2 heads = 4 channels per core)
    mask = np.triu(np.full((T, T), -30000.0, np.float32), 1)
    ident = np.eye(T, dtype=NPBF16)
    in_maps = []
    for c in cores:
        b, g = c // 4, c % 4
        sl = slice(4 * g, 4 * g + 4)
        in_maps.append({"qs": np.ascontiguousarray(q_all[b, sl]),
                        "ks": np.ascontiguousarray(k_all[b, sl]),
                        "vs": np.ascontiguousarray(v_all[b, sl]),
                        "mask": mask, "ident": ident})
    res2 = bass_utils.run_bass_kernel_spmd(_get("l2"), in_maps, core_ids=cores)

    y_all = np.empty((B, 16, T, HW), NPBF16)
    for c in cores:
        b, g = c // 4, c % 4
        ys = res2.results[c]["ys"]
        for p in range(2):
            y_all[b, 4 * g + 2 * p] = ys[p, 0:64]
            y_all[b, 4 * g + 2 * p + 1] = ys[p, 64:128]

    # ---- L3: output conv, image-sharded
    yimg = y_all.astype(np.float32).transpose(0, 2, 1, 3).reshape(IMGS, 16, H, W)
    l0o, l1o = _build_lhsT([wo], [bo])
    in_maps = [{"planes": _build_planes(yimg[c * IPC:(c + 1) * IPC]),
                "lhsT0": l0o, "lhsT1": l1o} for c in cores]
    res3 = bass_utils.run_bass_kernel_spmd(_get("l3"), in_maps, core_ids=cores)

    out = np.concatenate([_unpack_l3(res3.results[c]["out"]) for c in cores])
    return np.ascontiguousarray(out.reshape(B, T, O, H, W))


# revision 3
# speedup vs baseline: 1.1567x; 1.1567x over previous
"""Trainium2 Bass kernel for nn_CNNT_enhanced_denoising_runtime_53704271069472.

Computes, distributed across 8 NeuronCores:
    q/k/v = conv3x3(x, w?, b?)          (image-sharded: B*T=128 imgs, 16/core)
    att   = causal-softmax(q @ k^T / sqrt(D)) per (batch, head)
    y     = att @ v                      (head-sharded: 16 (b,head) pairs, 2/core)
    out   = conv3x3(y, wo, bo)           (image-sharded)

Three SPMD launches with host-side resharding between them. Convs are done as
matmuls over K = (3 kx-taps x 16 ch [+ ones bias row]) x 2 ky-rows = 97(+48)
against kx-pre-shifted zero-padded image planes built on the host; the 3x3
kernel's third ky row is a second accumulating matmul with an AP row offset.
Compute dtype bf16 (fp32 PSUM accumulation).
"""
import sys
import numpy as np

sys.path.insert(0, "/opt/trn_rl_repo")

import ml_dtypes  # noqa: E402
import concourse.bacc as bacc  # noqa: E402
import concourse.tile as tile  # noqa: E402
import concourse.bass as bass  # noqa: E402
from concourse import mybir, bass_utils  # noqa: E402

BF16 = mybir.dt.bfloat16
F32 = mybir.dt.float32
NPBF16 = ml_dtypes.bfloat16

B, T, C, H, W, O = 2, 64, 16, 128, 128, 16
HP, WP = H + 2, W + 2
HW = H * W
NH, HC = 8, 2
D = HC * HW
SCALE = float(1.0 / np.sqrt(np.float32(D)))
NCORES = 8
IMGS = B * T
IPC = IMGS // NCORES  # images per core
NPL = 98  # plane rows: 48 (ky0 kx-taps) + ones + 48 (ky1) + pad

_BUILD_CACHE = {}


# ---------------- device programs ----------------

def _build_l1():
    nc = bacc.Bacc("TRN2", target_bir_lowering=False, debug=False)
    planes = nc.dram_tensor("planes", (IPC, NPL, HP * WP), BF16, kind="ExternalInput")
    lhsT0 = nc.dram_tensor("lhsT0", (97, 48), BF16, kind="ExternalInput")
    lhsT1 = nc.dram_tensor("lhsT1", (48, 48), BF16, kind="ExternalInput")
    qkv = nc.dram_tensor("qkv_out", (IPC, 128, 8192), BF16, kind="ExternalOutput")

    with tile.TileContext(nc) as tc:
        with tc.tile_pool(name="w", bufs=1) as wpool, \
             tc.tile_pool(name="pl", bufs=2) as plpool, \
             tc.tile_pool(name="st", bufs=2) as stpool, \
             tc.tile_pool(name="ps", bufs=4, space="PSUM") as pspool:
            w0 = wpool.tile([97, 48], BF16, tag="w0")
            w1 = wpool.tile([48, 48], BF16, tag="w1")
            nc.sync.dma_start(w0[:], lhsT0.ap())
            nc.sync.dma_start(w1[:], lhsT1.ap())

            def rhs_view(pt, nrows, blk, ky):
                base = (blk * 4 + ky) * WP
                return pt[0:nrows, base:base + 4 * WP].rearrange(
                    "p (h w) -> p h w", w=WP)[:, :, 0:W]

            for img in range(IPC):
                pt = plpool.tile([NPL, HP * WP], BF16)
                nc.sync.dma_start(pt[:], planes.ap()[img])
                stage = stpool.tile([128, 8192], BF16)
                for blk2 in range(16):
                    ps = pspool.tile([128, 512], F32)
                    for half in range(2):
                        blk = blk2 * 2 + half
                        nc.tensor.matmul(ps[half * 64:half * 64 + 48, :],
                                         w0[:], rhs_view(pt, 97, blk, 0),
                                         start=True, stop=False,
                                         tile_position=(0, half * 64))
                        nc.tensor.matmul(ps[half * 64:half * 64 + 48, :],
                                         w1[:], rhs_view(pt, 48, blk, 2),
                                         start=False, stop=True,
                                         tile_position=(0, half * 64))
                    nc.vector.tensor_copy(stage[:, blk2 * 512:(blk2 + 1) * 512], ps[:])
                nc.sync.dma_start(qkv.ap()[img], stage[:])
    nc.compile()
    return nc


def _build_l2():
    nc = bacc.Bacc("TRN2", target_bir_lowering=False, debug=False)
    qks = nc.dram_tensor("qks", (8, T, HW), BF16, kind="ExternalInput")
    vs = nc.dram_tensor("vs", (4, T, HW), BF16, kind="ExternalInput")
    mask = nc.dram_tensor("mask", (T, T), F32, kind="ExternalInput")
    ident = nc.dram_tensor("ident", (T, T), BF16, kind="ExternalInput")
    ys = nc.dram_tensor("ys", (2, 128, HW), BF16, kind="ExternalOutput")

    with tile.TileContext(nc) as tc:
        with tc.tile_pool(name="cst", bufs=1) as cst, \
             tc.tile_pool(name="qk", bufs=3) as qkpool, \
             tc.tile_pool(name="sm", bufs=2) as smpool, \
             tc.tile_pool(name="v", bufs=3) as vpool, \
             tc.tile_pool(name="yst", bufs=2) as ypool, \
             tc.tile_pool(name="pst", bufs=1, space="PSUM") as pstpool, \
             tc.tile_pool(name="psy", bufs=3, space="PSUM") as psypool, \
             tc.tile_pool(name="psl", bufs=1, space="PSUM") as pslpool:
            mask_t = cst.tile([T, T], F32, tag="mask")
            nc.sync.dma_start(mask_t[:], mask.ap())
            id_t = cst.tile([T, T], BF16, tag="ident")
            nc.sync.dma_start(id_t[:], ident.ap())

            lg_ps = [pslpool.tile([128, 128], F32, tag=f"lg{h}", name=f"lg{h}")
                     for h in range(2)]
            for blk in range(128):
                qkt = qkpool.tile([128, 512], BF16, tag="qkT")
                src = qks.ap()[:, :, blk * 128:(blk + 1) * 128].rearrange("c t p -> (c t) p")
                nc.sync.dma_start_transpose(qkt[:], src)
                for h in range(2):
                    nc.tensor.matmul(lg_ps[h][:],
                                     qkt[:, h * 128:(h + 1) * 128],
                                     qkt[:, 256 + h * 128:256 + (h + 1) * 128],
                                     start=(blk == 0), stop=(blk == 127))

            attTs = []
            for h in range(2):
                lg = smpool.tile([T, T], F32, tag="lg")
                nc.vector.tensor_copy(lg[:], lg_ps[h][0:64, 0:64])
                nc.vector.tensor_add(lg[:], lg[:], lg_ps[h][64:128, 64:128])
                nc.vector.tensor_scalar(lg[:], lg[:], SCALE, None,
                                        op0=mybir.AluOpType.mult)
                nc.vector.tensor_add(lg[:], lg[:], mask_t[:])
                mx = smpool.tile([T, 1], F32, tag="mx")
                nc.vector.reduce_max(mx[:], lg[:], axis=mybir.AxisListType.X, negate=True)
                e = smpool.tile([T, T], F32, tag="e")
                sm_acc = smpool.tile([T, 1], F32, tag="smacc")
                nc.scalar.activation(e[:], lg[:], mybir.ActivationFunctionType.Exp,
                                     bias=mx[:], scale=1.0, accum_out=sm_acc[:])
                rc = smpool.tile([T, 1], F32, tag="rc")
                nc.vector.reciprocal(rc[:], sm_acc[:])
                att = smpool.tile([T, T], BF16, tag="att")
                nc.vector.tensor_scalar(att[:], e[:], rc[:], None,
                                        op0=mybir.AluOpType.mult)
                ps_t = pstpool.tile([T, T], BF16, tag="pst")
                nc.tensor.transpose(ps_t[:], att[:], id_t[:])
                attT = smpool.tile([128, T], BF16, tag=f"attT{h}", name=f"attT{h}")
                nc.vector.tensor_copy(attT[0:64, :], ps_t[:])
                nc.vector.tensor_copy(attT[64:128, :], ps_t[:])
                attTs.append(attT)

            for p in range(2):
                yst = ypool.tile([128, HW], BF16, tag="yst")
                for vb in range(4):
                    vt = vpool.tile([128, 4096], BF16, tag="vt")
                    src_v = vs.ap()[2 * p:2 * p + 2, :, vb * 4096:(vb + 1) * 4096]
                    nc.sync.dma_start(vt[:], src_v.rearrange("c t p -> (c t) p"))
                    for ci in range(2):
                        attT = attTs[p]
                        for j in range(8):
                            ps_y = psypool.tile([T, 512], F32, tag="psy")
                            nc.tensor.matmul(ps_y[:], attT[ci * 64:ci * 64 + 64, :],
                                             vt[ci * 64:ci * 64 + 64, j * 512:(j + 1) * 512],
                                             start=True, stop=True)
                            col = vb * 4096 + j * 512
                            nc.vector.tensor_copy(
                                yst[ci * 64:ci * 64 + 64, col:col + 512], ps_y[:])
                nc.sync.dma_start(ys.ap()[p], yst[:])
    nc.compile()
    return nc


def _build_l3():
    nc = bacc.Bacc("TRN2", target_bir_lowering=False, debug=False)
    planes = nc.dram_tensor("planes", (IPC, NPL, HP * WP), BF16, kind="ExternalInput")
    lhsT0 = nc.dram_tensor("lhsT0", (97, 16), BF16, kind="ExternalInput")
    lhsT1 = nc.dram_tensor("lhsT1", (48, 16), BF16, kind="ExternalInput")
    out = nc.dram_tensor("out", (IPC, 128, 4096), F32, kind="ExternalOutput")

    with tile.TileContext(nc) as tc:
        with tc.tile_pool(name="w", bufs=1) as wpool, \
             tc.tile_pool(name="pl", bufs=2) as plpool, \
             tc.tile_pool(name="st", bufs=2) as stpool, \
             tc.tile_pool(name="ps", bufs=4, space="PSUM") as pspool:
            w0 = wpool.tile([97, 16], BF16, tag="w0")
            w1 = wpool.tile([48, 16], BF16, tag="w1")
            nc.sync.dma_start(w0[:], lhsT0.ap())
            nc.sync.dma_start(w1[:], lhsT1.ap())

            def rhs_view(pt, nrows, blk, ky):
                base = (blk * 4 + ky) * WP
                return pt[0:nrows, base:base + 4 * WP].rearrange(
                    "p (h w) -> p h w", w=WP)[:, :, 0:W]

            for img in range(IPC):
                pt = plpool.tile([NPL, HP * WP], BF16)
                nc.sync.dma_start(pt[:], planes.ap()[img])
                stage = stpool.tile([128, 4096], F32)
                for blk2 in range(16):
                    ps = pspool.tile([128, 512], F32)
                    for half in range(2):
                        blk = blk2 * 2 + half
                        po = half * 64
                        nc.tensor.matmul(ps[po:po + 16, :], w0[:],
                                         rhs_view(pt, 97, blk, 0),
                                         start=True, stop=False,
                                         tile_position=(0, po))
                        nc.tensor.matmul(ps[po:po + 16, :], w1[:],
                                         rhs_view(pt, 48, blk, 2),
                                         start=False, stop=True,
                                         tile_position=(0, po))
                    for half in range(2):
                        slot = 2 * (blk2 % 2) + half
                        nc.vector.tensor_copy(
                            stage[32 * slot:32 * slot + 16,
                                  (blk2 // 2) * 512:(blk2 // 2) * 512 + 512],
                            ps[half * 64:half * 64 + 16, :])
                nc.sync.dma_start(out.ap()[img], stage[:])
    nc.compile()
    return nc


def _get(name):
    if name not in _BUILD_CACHE:
        _BUILD_CACHE[name] = {"l1": _build_l1, "l2": _build_l2, "l3": _build_l3}[name]()
    return _BUILD_CACHE[name]


# ---------------- host-side packing ----------------

def _build_planes(imgs_chw):
    """imgs_chw: [N, 16, H, W] float32-like -> [N, 98, HP*WP] bf16."""
    N = imgs_chw.shape[0]
    xpad = np.zeros((N, C, HP, WP), np.float32)
    xpad[:, :, 1:H + 1, 1:W + 1] = imgs_chw.astype(np.float32)
    flat = xpad.reshape(N, C, HP * WP)
    p = np.zeros((N, NPL, HP * WP), np.float32)
    p[:, 0:16] = flat
    p[:, 16:32, :-1] = flat[:, :, 1:]
    p[:, 32:48, :-2] = flat[:, :, 2:]
    p[:, 48] = 1.0
    p[:, 49:97, :-WP] = p[:, 0:48, WP:]
    return p.astype(NPBF16)


def _build_lhsT(ws, bs):
    """ws: list of [O,C,3,3]; bs: list of [O] -> lhsT0 [97, 16*len], lhsT1 [48, 16*len]."""
    n = len(ws)
    m = np.zeros((3, 49, 16 * n), np.float32)
    for j, (w, b) in enumerate(zip(ws, bs)):
        for ky in range(3):
            for kx in range(3):
                m[ky, kx * 16:(kx + 1) * 16, j * 16:(j + 1) * 16] = w[:, :, ky, kx].T
        m[1, 48, j * 16:(j + 1) * 16] = b
    l0 = np.zeros((97, 16 * n), np.float32)
    l0[0:48] = m[0][0:48]
    l0[48] = m[1][48]
    l0[49:97] = m[1][0:48]
    return l0.astype(NPBF16), m[2][0:48].astype(NPBF16)


def _unpack_qkv(qkv_out):
    """[N,128,8192] bf16 -> q,k,v each [N,16,HW]."""
    N = qkv_out.shape[0]
    s = qkv_out.reshape(N, 128, 16, 512)
    out = np.empty((N, 48, 32, 512), qkv_out.dtype)
    out[:, :, 0::2] = s[:, 0:48]
    out[:, :, 1::2] = s[:, 64:112]
    out = out.reshape(N, 48, HW)
    return out[:, 0:16], out[:, 16:32], out[:, 32:48]


def _unpack_l3(o):
    """[N,128,4096] f32 -> [N,16,HW].

    blk = blk2*2 + half lives at stage rows 32*(2*(blk2%2)+half)+c,
    cols (blk2//2)*512 : +512."""
    N = o.shape[0]
    s = o.reshape(N, 4, 32, 8, 512)      # [N, slot, 32row, blk2//2, 512]
    s = s[:, :, 0:16]                    # -> [N, slot, c, blk2h, 512]
    # blk index: blk2 = 2*(blk2h) + slot//2 ... slot = 2*(blk2%2)+half
    out = np.empty((N, 16, 32, 512), o.dtype)
    for slot in range(4):
        half, b2lo = slot % 2, slot // 2
        for b2h in range(8):
            blk2 = 2 * b2h + b2lo
            blk = blk2 * 2 + half
            out[:, :, blk] = s[:, slot, :, b2h]
    return np.ascontiguousarray(out).reshape(N, 16, HW)


# ---------------- top level ----------------

def kernel(x, wq, bq, wk, bk, wv, bv, wo, bo):
    x, wq, bq, wk, bk, wv, bv, wo, bo = (
        np.asarray(a, np.float32) for a in (x, wq, bq, wk, bk, wv, bv, wo, bo))
    ximg = x.reshape(IMGS, C, H, W)
    cores = list(range(NCORES))

    # ---- L1: q/k/v convs, image-sharded
    l0, l1 = _build_lhsT([wq, wk, wv], [bq, bk, bv])
    in_maps = [{"planes": _build_planes(ximg[c * IPC:(c + 1) * IPC]),
                "lhsT0": l0, "lhsT1": l1} for c in cores]
    res1 = bass_utils.run_bass_kernel_spmd(_get("l1"), in_maps, core_ids=cores)

    # assemble channel-major [B, 16, T, HW] bf16
    q_all = np.empty((B, 16, T, HW), NPBF16)
    k_all = np.empty_like(q_all)
    v_all = np.empty_like(q_all)
    for c in cores:
        q, k, v = _unpack_qkv(res1.results[c]["qkv_out"])
        b0 = (c * IPC) // T
        t0 = (c * IPC) % T
        q_all[b0, :, t0:t0 + IPC] = q.transpose(1, 0, 2)
        k_all[b0, :, t0:t0 + IPC] = k.transpose(1, 0, 2)
        v_all[b0, :, t0:t0 + IPC] = v.transpose(1, 0, 2)

    # ---- L2: attention, head-sharded (2 heads = 4 channels per core)
    mask = np.triu(np.full((T, T), -30000.0, np.float32), 1)
    ident = np.eye(T, dtype=NPBF16)
    in_maps = []
    for c in cores:
        b, g = c // 4, c % 4
        sl = slice(4 * g, 4 * g + 4)
        qks = np.concatenate([q_all[b, sl], k_all[b, sl]], axis=0)
        in_maps.append({"qks": np.ascontiguousarray(qks),
                        "vs": np.ascontiguousarray(v_all[b, sl]),
                        "mask": mask, "ident": ident})
    res2 = bass_utils.run_bass_kernel_spmd(_get("l2"), in_maps, core_ids=cores)

    y_all = np.empty((B, 16, T, HW), NPBF16)
    for c in cores:
        b, g = c // 4, c % 4
        ys = res2.results[c]["ys"]
        for p in range(2):
            y_all[b, 4 * g + 2 * p] = ys[p, 0:64]
            y_all[b, 4 * g + 2 * p + 1] = ys[p, 64:128]

    # ---- L3: output conv, image-sharded
    yimg = y_all.astype(np.float32).transpose(0, 2, 1, 3).reshape(IMGS, 16, H, W)
    l0o, l1o = _build_lhsT([wo], [bo])
    in_maps = [{"planes": _build_planes(yimg[c * IPC:(c + 1) * IPC]),
                "lhsT0": l0o, "lhsT1": l1o} for c in cores]
    res3 = bass_utils.run_bass_kernel_spmd(_get("l3"), in_maps, core_ids=cores)

    out = np.concatenate([_unpack_l3(res3.results[c]["out"]) for c in cores])
    return np.ascontiguousarray(out.reshape(B, T, O, H, W))


# revision 5
# speedup vs baseline: 1.2842x; 1.1102x over previous
"""Trainium2 Bass kernel for nn_CNNT_enhanced_denoising_runtime_53704271069472.

Computes, distributed across 8 NeuronCores:
    q/k/v = conv3x3(x, w?, b?)          (image-sharded: B*T=128 imgs, 16/core)
    att   = causal-softmax(q @ k^T / sqrt(D)) per (batch, head)
    y     = att @ v                      (head-sharded: 16 (b,head) pairs, 2/core)
    out   = conv3x3(y, wo, bo)           (image-sharded)

Three SPMD launches with host-side resharding between them. Convs are done as
matmuls over K = (3 kx-taps x 16 ch [+ ones bias row]) x 2 ky-rows = 97(+48)
against kx-pre-shifted zero-padded image planes built on the host; the 3x3
kernel's third ky row is a second accumulating matmul with an AP row offset.
Compute dtype bf16 (fp32 PSUM accumulation).
"""
import sys
import numpy as np

sys.path.insert(0, "/opt/trn_rl_repo")

import ml_dtypes  # noqa: E402
import concourse.bacc as bacc  # noqa: E402
import concourse.tile as tile  # noqa: E402
import concourse.bass as bass  # noqa: E402
from concourse import mybir, bass_utils  # noqa: E402

BF16 = mybir.dt.bfloat16
F32 = mybir.dt.float32
NPBF16 = ml_dtypes.bfloat16

B, T, C, H, W, O = 2, 64, 16, 128, 128, 16
HP, WP = H + 2, W + 2
HW = H * W
NH, HC = 8, 2
D = HC * HW
SCALE = float(1.0 / np.sqrt(np.float32(D)))
NCORES = 8
IMGS = B * T
IPC = IMGS // NCORES  # images per core
NPL = 98  # plane rows: 48 (ky0 kx-taps) + ones + 48 (ky1) + pad

_BUILD_CACHE = {}


# ---------------- device programs ----------------

def _build_l1():
    nc = bacc.Bacc("TRN2", target_bir_lowering=False, debug=False)
    planes = nc.dram_tensor("planes", (IPC, NPL, HP * WP), BF16, kind="ExternalInput")
    lhsT0 = nc.dram_tensor("lhsT0", (97, 48), BF16, kind="ExternalInput")
    lhsT1 = nc.dram_tensor("lhsT1", (48, 48), BF16, kind="ExternalInput")
    qkv = nc.dram_tensor("qkv_out", (IPC, 128, 8192), BF16, kind="ExternalOutput")

    with tile.TileContext(nc) as tc:
        with tc.tile_pool(name="w", bufs=1) as wpool, \
             tc.tile_pool(name="pl", bufs=3) as plpool, \
             tc.tile_pool(name="st", bufs=3) as stpool, \
             tc.tile_pool(name="ps", bufs=4, space="PSUM") as pspool:
            w0 = wpool.tile([97, 48], BF16, tag="w0")
            w1 = wpool.tile([48, 48], BF16, tag="w1")
            nc.sync.dma_start(w0[:], lhsT0.ap())
            nc.sync.dma_start(w1[:], lhsT1.ap())

            def rhs_view(pt, nrows, blk, ky):
                base = (blk * 4 + ky) * WP
                return pt[0:nrows, base:base + 4 * WP].rearrange(
                    "p (h w) -> p h w", w=WP)[:, :, 0:W]

            for img in range(IPC):
                pt = plpool.tile([NPL, HP * WP], BF16)
                nc.scalar.dma_start(pt[:], planes.ap()[img])
                stage = stpool.tile([128, 8192], BF16)
                for blk2 in range(16):
                    ps = pspool.tile([128, 512], F32)
                    for half in range(2):
                        blk = blk2 * 2 + half
                        nc.tensor.matmul(ps[half * 64:half * 64 + 48, :],
                                         w0[:], rhs_view(pt, 97, blk, 0),
                                         start=True, stop=False,
                                         tile_position=(0, half * 64))
                        nc.tensor.matmul(ps[half * 64:half * 64 + 48, :],
                                         w1[:], rhs_view(pt, 48, blk, 2),
                                         start=False, stop=True,
                                         tile_position=(0, half * 64))
                    nc.vector.tensor_copy(stage[:, blk2 * 512:(blk2 + 1) * 512], ps[:])
                nc.sync.dma_start(qkv.ap()[img], stage[:])
    nc.compile()
    return nc


def _build_l2():
    nc = bacc.Bacc("TRN2", target_bir_lowering=False, debug=False)
    qks = nc.dram_tensor("qks", (8, T, HW), BF16, kind="ExternalInput")
    vs = nc.dram_tensor("vs", (4, T, HW), BF16, kind="ExternalInput")
    mask = nc.dram_tensor("mask", (T, T), F32, kind="ExternalInput")
    ident = nc.dram_tensor("ident", (T, T), BF16, kind="ExternalInput")
    ys = nc.dram_tensor("ys", (2, 128, HW), BF16, kind="ExternalOutput")

    with tile.TileContext(nc) as tc:
        with tc.tile_pool(name="cst", bufs=1) as cst, \
             tc.tile_pool(name="qk", bufs=4) as qkpool, \
             tc.tile_pool(name="sm", bufs=2) as smpool, \
             tc.tile_pool(name="v", bufs=8) as vpool, \
             tc.tile_pool(name="yst", bufs=2) as ypool, \
             tc.tile_pool(name="pst", bufs=1, space="PSUM") as pstpool, \
             tc.tile_pool(name="psy", bufs=3, space="PSUM") as psypool, \
             tc.tile_pool(name="psl", bufs=1, space="PSUM") as pslpool:
            mask_t = cst.tile([T, T], F32, tag="mask")
            nc.sync.dma_start(mask_t[:], mask.ap())
            id_t = cst.tile([T, T], BF16, tag="ident")
            nc.sync.dma_start(id_t[:], ident.ap())

            lg_ps = [pslpool.tile([128, 128], F32, tag=f"lg{h}", name=f"lg{h}")
                     for h in range(2)]
            for blk in range(128):
                qkt = qkpool.tile([128, 512], BF16, tag="qkT")
                src = qks.ap()[:, :, blk * 128:(blk + 1) * 128].rearrange("c t p -> (c t) p")
                nc.sync.dma_start_transpose(qkt[:], src)
                for h in range(2):
                    nc.tensor.matmul(lg_ps[h][:],
                                     qkt[:, h * 128:(h + 1) * 128],
                                     qkt[:, 256 + h * 128:256 + (h + 1) * 128],
                                     start=(blk == 0), stop=(blk == 127))

            attTs = []
            for h in range(2):
                lg = smpool.tile([T, T], F32, tag="lg")
                nc.vector.tensor_copy(lg[:], lg_ps[h][0:64, 0:64])
                nc.vector.tensor_add(lg[:], lg[:], lg_ps[h][64:128, 64:128])
                nc.vector.tensor_scalar(lg[:], lg[:], SCALE, None,
                                        op0=mybir.AluOpType.mult)
                nc.vector.tensor_add(lg[:], lg[:], mask_t[:])
                mx = smpool.tile([T, 1], F32, tag="mx")
                nc.vector.reduce_max(mx[:], lg[:], axis=mybir.AxisListType.X, negate=True)
                e = smpool.tile([T, T], F32, tag="e")
                sm_acc = smpool.tile([T, 1], F32, tag="smacc")
                nc.scalar.activation(e[:], lg[:], mybir.ActivationFunctionType.Exp,
                                     bias=mx[:], scale=1.0, accum_out=sm_acc[:])
                rc = smpool.tile([T, 1], F32, tag="rc")
                nc.vector.reciprocal(rc[:], sm_acc[:])
                att = smpool.tile([T, T], BF16, tag="att")
                nc.vector.tensor_scalar(att[:], e[:], rc[:], None,
                                        op0=mybir.AluOpType.mult)
                ps_t = pstpool.tile([T, T], BF16, tag="pst")
                nc.tensor.transpose(ps_t[:], att[:], id_t[:])
                attT = smpool.tile([128, T], BF16, tag=f"attT{h}", name=f"attT{h}")
                nc.vector.tensor_copy(attT[0:64, :], ps_t[:])
                nc.vector.tensor_copy(attT[64:128, :], ps_t[:])
                attTs.append(attT)

            for p in range(2):
                yst = ypool.tile([128, HW], BF16, tag="yst")
                for vb in range(4):
                    vt = vpool.tile([128, 4096], BF16, tag="vt")
                    src_v = vs.ap()[2 * p:2 * p + 2, :, vb * 4096:(vb + 1) * 4096]
                    nc.scalar.dma_start(vt[:], src_v.rearrange("c t p -> (c t) p"))
                    for ci in range(2):
                        attT = attTs[p]
                        for j in range(8):
                            ps_y = psypool.tile([T, 512], F32, tag="psy")
                            nc.tensor.matmul(ps_y[:], attT[ci * 64:ci * 64 + 64, :],
                                             vt[ci * 64:ci * 64 + 64, j * 512:(j + 1) * 512],
                                             start=True, stop=True)
                            col = vb * 4096 + j * 512
                            nc.vector.tensor_copy(
                                yst[ci * 64:ci * 64 + 64, col:col + 512], ps_y[:])
                nc.sync.dma_start(ys.ap()[p], yst[:])
    nc.compile()
    return nc


def _build_l3():
    nc = bacc.Bacc("TRN2", target_bir_lowering=False, debug=False)
    planes = nc.dram_tensor("planes", (IPC, NPL, HP * WP), BF16, kind="ExternalInput")
    lhsT0 = nc.dram_tensor("lhsT0", (97, 16), BF16, kind="ExternalInput")
    lhsT1 = nc.dram_tensor("lhsT1", (48, 16), BF16, kind="ExternalInput")
    out = nc.dram_tensor("out", (IPC, 128, 4096), F32, kind="ExternalOutput")

    with tile.TileContext(nc) as tc:
        with tc.tile_pool(name="w", bufs=1) as wpool, \
             tc.tile_pool(name="pl", bufs=3) as plpool, \
             tc.tile_pool(name="st", bufs=3) as stpool, \
             tc.tile_pool(name="ps", bufs=4, space="PSUM") as pspool:
            w0 = wpool.tile([97, 16], BF16, tag="w0")
            w1 = wpool.tile([48, 16], BF16, tag="w1")
            nc.sync.dma_start(w0[:], lhsT0.ap())
            nc.sync.dma_start(w1[:], lhsT1.ap())

            def rhs_view(pt, nrows, blk, ky):
                base = (blk * 4 + ky) * WP
                return pt[0:nrows, base:base + 4 * WP].rearrange(
                    "p (h w) -> p h w", w=WP)[:, :, 0:W]

            for img in range(IPC):
                pt = plpool.tile([NPL, HP * WP], BF16)
                nc.scalar.dma_start(pt[:], planes.ap()[img])
                stage = stpool.tile([128, 4096], F32)
                for blk2 in range(16):
                    ps = pspool.tile([128, 512], F32)
                    for half in range(2):
                        blk = blk2 * 2 + half
                        po = half * 64
                        nc.tensor.matmul(ps[po:po + 16, :], w0[:],
                                         rhs_view(pt, 97, blk, 0),
                                         start=True, stop=False,
                                         tile_position=(0, po))
                        nc.tensor.matmul(ps[po:po + 16, :], w1[:],
                                         rhs_view(pt, 48, blk, 2),
                                         start=False, stop=True,
                                         tile_position=(0, po))
                    for half in range(2):
                        slot = 2 * (blk2 % 2) + half
                        nc.vector.tensor_copy(
                            stage[32 * slot:32 * slot + 16,
                                  (blk2 // 2) * 512:(blk2 // 2) * 512 + 512],
                            ps[half * 64:half * 64 + 16, :])
                nc.sync.dma_start(out.ap()[img], stage[:])
    nc.compile()
    return nc


def _get(name):
    if name not in _BUILD_CACHE:
        _BUILD_CACHE[name] = {"l1": _build_l1, "l2": _build_l2, "l3": _build_l3}[name]()
    return _BUILD_CACHE[name]


# ---------------- host-side packing ----------------

def _build_planes(imgs_chw):
    """imgs_chw: [N, 16, H, W] float32-like -> [N, 98, HP*WP] bf16."""
    N = imgs_chw.shape[0]
    xpad = np.zeros((N, C, HP, WP), np.float32)
    xpad[:, :, 1:H + 1, 1:W + 1] = imgs_chw.astype(np.float32)
    flat = xpad.reshape(N, C, HP * WP)
    p = np.zeros((N, NPL, HP * WP), np.float32)
    p[:, 0:16] = flat
    p[:, 16:32, :-1] = flat[:, :, 1:]
    p[:, 32:48, :-2] = flat[:, :, 2:]
    p[:, 48] = 1.0
    p[:, 49:97, :-WP] = p[:, 0:48, WP:]
    return p.astype(NPBF16)


def _build_lhsT(ws, bs):
    """ws: list of [O,C,3,3]; bs: list of [O] -> lhsT0 [97, 16*len], lhsT1 [48, 16*len]."""
    n = len(ws)
    m = np.zeros((3, 49, 16 * n), np.float32)
    for j, (w, b) in enumerate(zip(ws, bs)):
        for ky in range(3):
            for kx in range(3):
                m[ky, kx * 16:(kx + 1) * 16, j * 16:(j + 1) * 16] = w[:, :, ky, kx].T
        m[1, 48, j * 16:(j + 1) * 16] = b
    l0 = np.zeros((97, 16 * n), np.float32)
    l0[0:48] = m[0][0:48]
    l0[48] = m[1][48]
    l0[49:97] = m[1][0:48]
    return l0.astype(NPBF16), m[2][0:48].astype(NPBF16)


def _unpack_qkv(qkv_out):
    """[N,128,8192] bf16 -> q,k,v each [N,16,HW]."""
    N = qkv_out.shape[0]
    s = qkv_out.reshape(N, 128, 16, 512)
    out = np.empty((N, 48, 32, 512), qkv_out.dtype)
    out[:, :, 0::2] = s[:, 0:48]
    out[:, :, 1::2] = s[:, 64:112]
    out = out.reshape(N, 48, HW)
    return out[:, 0:16], out[:, 16:32], out[:, 32:48]


def _unpack_l3(o):
    """[N,128,4096] f32 -> [N,16,HW].

    blk = blk2*2 + half lives at stage rows 32*(2*(blk2%2)+half)+c,
    cols (blk2//2)*512 : +512."""
    N = o.shape[0]
    s = o.reshape(N, 4, 32, 8, 512)      # [N, slot, 32row, blk2//2, 512]
    s = s[:, :, 0:16]                    # -> [N, slot, c, blk2h, 512]
    # blk index: blk2 = 2*(blk2h) + slot//2 ... slot = 2*(blk2%2)+half
    out = np.empty((N, 16, 32, 512), o.dtype)
    for slot in range(4):
        half, b2lo = slot % 2, slot // 2
        for b2h in range(8):
            blk2 = 2 * b2h + b2lo
            blk = blk2 * 2 + half
            out[:, :, blk] = s[:, slot, :, b2h]
    return np.ascontiguousarray(out).reshape(N, 16, HW)


# ---------------- top level ----------------

def kernel(x, wq, bq, wk, bk, wv, bv, wo, bo):
    x, wq, bq, wk, bk, wv, bv, wo, bo = (
        np.asarray(a, np.float32) for a in (x, wq, bq, wk, bk, wv, bv, wo, bo))
    ximg = x.reshape(IMGS, C, H, W)
    cores = list(range(NCORES))

    # ---- L1: q/k/v convs, image-sharded
    l0, l1 = _build_lhsT([wq, wk, wv], [bq, bk, bv])
    in_maps = [{"planes": _build_planes(ximg[c * IPC:(c + 1) * IPC]),
                "lhsT0": l0, "lhsT1": l1} for c in cores]
    res1 = bass_utils.run_bass_kernel_spmd(_get("l1"), in_maps, core_ids=cores)

    # assemble channel-major [B, 16, T, HW] bf16
    q_all = np.empty((B, 16, T, HW), NPBF16)
    k_all = np.empty_like(q_all)
    v_all = np.empty_like(q_all)
    for c in cores:
        q, k, v = _unpack_qkv(res1.results[c]["qkv_out"])
        b0 = (c * IPC) // T
        t0 = (c * IPC) % T
        q_all[b0, :, t0:t0 + IPC] = q.transpose(1, 0, 2)
        k_all[b0, :, t0:t0 + IPC] = k.transpose(1, 0, 2)
        v_all[b0, :, t0:t0 + IPC] = v.transpose(1, 0, 2)

    # ---- L2: attention, head-sharded (2 heads = 4 channels per core)
    mask = np.triu(np.full((T, T), -30000.0, np.float32), 1)
    ident = np.eye(T, dtype=NPBF16)
    in_maps = []
    for c in cores:
        b, g = c // 4, c % 4
        sl = slice(4 * g, 4 * g + 4)
        qks = np.concatenate([q_all[b, sl], k_all[b, sl]], axis=0)
        in_maps.append({"qks": np.ascontiguousarray(qks),
                        "vs": np.ascontiguousarray(v_all[b, sl]),
                        "mask": mask, "ident": ident})
    res2 = bass_utils.run_bass_kernel_spmd(_get("l2"), in_maps, core_ids=cores)

    y_all = np.empty((B, 16, T, HW), NPBF16)
    for c in cores:
        b, g = c // 4, c % 4
        ys = res2.results[c]["ys"]
        for p in range(2):
            y_all[b, 4 * g + 2 * p] = ys[p, 0:64]
            y_all[b, 4 * g + 2 * p + 1] = ys[p, 64:128]

    # ---- L3: output conv, image-sharded
    yimg = y_all.astype(np.float32).transpose(0, 2, 1, 3).reshape(IMGS, 16, H, W)
    l0o, l1o = _build_lhsT([wo], [bo])
    in_maps = [{"planes": _build_planes(yimg[c * IPC:(c + 1) * IPC]),
                "lhsT0": l0o, "lhsT1": l1o} for c in cores]
    res3 = bass_utils.run_bass_kernel_spmd(_get("l3"), in_maps, core_ids=cores)

    out = np.concatenate([_unpack_l3(res3.results[c]["out"]) for c in cores])
    return np.ascontiguousarray(out.reshape(B, T, O, H, W))


# revision 6
# speedup vs baseline: 1.3291x; 1.0349x over previous
"""Trainium2 Bass kernel for nn_CNNT_enhanced_denoising_runtime_53704271069472.

Computes, distributed across 8 NeuronCores:
    q/k/v = conv3x3(x, w?, b?)          (image-sharded: B*T=128 imgs, 16/core)
    att   = causal-softmax(q @ k^T / sqrt(D)) per (batch, head)
    y     = att @ v                      (head-sharded: 16 (b,head) pairs, 2/core)
    out   = conv3x3(y, wo, bo)           (image-sharded)

Three SPMD launches with host-side resharding between them. Convs are done as
matmuls over K = (3 kx-taps x 16 ch [+ ones bias row]) x 2 ky-rows = 97(+48)
against kx-pre-shifted zero-padded image planes built on the host; the 3x3
kernel's third ky row is a second accumulating matmul with an AP row offset.
Compute dtype bf16 (fp32 PSUM accumulation).
"""
import sys
import numpy as np

sys.path.insert(0, "/opt/trn_rl_repo")

import ml_dtypes  # noqa: E402
import concourse.bacc as bacc  # noqa: E402
import concourse.tile as tile  # noqa: E402
import concourse.bass as bass  # noqa: E402
from concourse import mybir, bass_utils  # noqa: E402

BF16 = mybir.dt.bfloat16
F32 = mybir.dt.float32
NPBF16 = ml_dtypes.bfloat16

B, T, C, H, W, O = 2, 64, 16, 128, 128, 16
HP, WP = H + 2, W + 2
HW = H * W
NH, HC = 8, 2
D = HC * HW
SCALE = float(1.0 / np.sqrt(np.float32(D)))
NCORES = 8
IMGS = B * T
IPC = IMGS // NCORES  # images per core
NPL = 98  # plane rows: 48 (ky0 kx-taps) + ones + 48 (ky1) + pad

_BUILD_CACHE = {}


# ---------------- device programs ----------------

def _build_l1():
    nc = bacc.Bacc("TRN2", target_bir_lowering=False, debug=False)
    planes = nc.dram_tensor("planes", (IPC, NPL, HP * WP), BF16, kind="ExternalInput")
    lhsT0 = nc.dram_tensor("lhsT0", (97, 48), BF16, kind="ExternalInput")
    lhsT1 = nc.dram_tensor("lhsT1", (48, 48), BF16, kind="ExternalInput")
    qkv = nc.dram_tensor("qkv_out", (IPC, 128, 8192), BF16, kind="ExternalOutput")

    with tile.TileContext(nc) as tc:
        with tc.tile_pool(name="w", bufs=1) as wpool, \
             tc.tile_pool(name="pl", bufs=3) as plpool, \
             tc.tile_pool(name="st", bufs=3) as stpool, \
             tc.tile_pool(name="ps", bufs=4, space="PSUM") as pspool:
            w0 = wpool.tile([97, 48], BF16, tag="w0")
            w1 = wpool.tile([48, 48], BF16, tag="w1")
            nc.sync.dma_start(w0[:], lhsT0.ap())
            nc.sync.dma_start(w1[:], lhsT1.ap())

            def rhs_view(pt, nrows, blk, ky):
                base = (blk * 4 + ky) * WP
                return pt[0:nrows, base:base + 4 * WP].rearrange(
                    "p (h w) -> p h w", w=WP)[:, :, 0:W]

            for img in range(IPC):
                pt = plpool.tile([NPL, HP * WP], BF16)
                nc.scalar.dma_start(pt[:], planes.ap()[img])
                stage = stpool.tile([128, 8192], BF16)
                for q4 in range(8):
                    ps = pspool.tile([128, 1024], F32)
                    for sub in range(2):
                        for half in range(2):
                            blk = q4 * 4 + sub * 2 + half
                            psv = ps[half * 64:half * 64 + 48,
                                     sub * 512:sub * 512 + 512]
                            nc.tensor.matmul(psv, w0[:], rhs_view(pt, 97, blk, 0),
                                             start=True, stop=False,
                                             tile_position=(0, half * 64))
                            nc.tensor.matmul(psv, w1[:], rhs_view(pt, 48, blk, 2),
                                             start=False, stop=True,
                                             tile_position=(0, half * 64))
                    nc.vector.tensor_copy(stage[:, q4 * 1024:(q4 + 1) * 1024], ps[:])
                nc.sync.dma_start(qkv.ap()[img], stage[:])
    nc.compile()
    return nc


def _build_l2():
    nc = bacc.Bacc("TRN2", target_bir_lowering=False, debug=False)
    qks = nc.dram_tensor("qks", (8, T, HW), BF16, kind="ExternalInput")
    vs = nc.dram_tensor("vs", (4, T, HW), BF16, kind="ExternalInput")
    mask = nc.dram_tensor("mask", (T, T), F32, kind="ExternalInput")
    ident = nc.dram_tensor("ident", (T, T), BF16, kind="ExternalInput")
    ys = nc.dram_tensor("ys", (2, 128, HW), BF16, kind="ExternalOutput")

    with tile.TileContext(nc) as tc:
        with tc.tile_pool(name="cst", bufs=1) as cst, \
             tc.tile_pool(name="qk", bufs=4) as qkpool, \
             tc.tile_pool(name="sm", bufs=2) as smpool, \
             tc.tile_pool(name="v", bufs=8) as vpool, \
             tc.tile_pool(name="yst", bufs=2) as ypool, \
             tc.tile_pool(name="pst", bufs=1, space="PSUM") as pstpool, \
             tc.tile_pool(name="psy", bufs=3, space="PSUM") as psypool, \
             tc.tile_pool(name="psl", bufs=1, space="PSUM") as pslpool:
            mask_t = cst.tile([T, T], F32, tag="mask")
            nc.sync.dma_start(mask_t[:], mask.ap())
            id_t = cst.tile([T, T], BF16, tag="ident")
            nc.sync.dma_start(id_t[:], ident.ap())

            lg_ps = [pslpool.tile([128, 128], F32, tag=f"lg{h}", name=f"lg{h}")
                     for h in range(2)]
            for blk in range(128):
                qkt = qkpool.tile([128, 512], BF16, tag="qkT")
                src = qks.ap()[:, :, blk * 128:(blk + 1) * 128].rearrange("c t p -> (c t) p")
                nc.sync.dma_start_transpose(qkt[:], src)
                for h in range(2):
                    nc.tensor.matmul(lg_ps[h][:],
                                     qkt[:, h * 128:(h + 1) * 128],
                                     qkt[:, 256 + h * 128:256 + (h + 1) * 128],
                                     start=(blk == 0), stop=(blk == 127))

            attTs = []
            for h in range(2):
                lg = smpool.tile([T, T], F32, tag="lg")
                nc.vector.tensor_copy(lg[:], lg_ps[h][0:64, 0:64])
                nc.vector.tensor_add(lg[:], lg[:], lg_ps[h][64:128, 64:128])
                nc.vector.tensor_scalar(lg[:], lg[:], SCALE, None,
                                        op0=mybir.AluOpType.mult)
                nc.vector.tensor_add(lg[:], lg[:], mask_t[:])
                mx = smpool.tile([T, 1], F32, tag="mx")
                nc.vector.reduce_max(mx[:], lg[:], axis=mybir.AxisListType.X, negate=True)
                e = smpool.tile([T, T], F32, tag="e")
                sm_acc = smpool.tile([T, 1], F32, tag="smacc")
                nc.scalar.activation(e[:], lg[:], mybir.ActivationFunctionType.Exp,
                                     bias=mx[:], scale=1.0, accum_out=sm_acc[:])
                rc = smpool.tile([T, 1], F32, tag="rc")
                nc.vector.reciprocal(rc[:], sm_acc[:])
                att = smpool.tile([T, T], BF16, tag="att")
                nc.vector.tensor_scalar(att[:], e[:], rc[:], None,
                                        op0=mybir.AluOpType.mult)
                ps_t = pstpool.tile([T, T], BF16, tag="pst")
                nc.tensor.transpose(ps_t[:], att[:], id_t[:])
                attT = smpool.tile([128, T], BF16, tag=f"attT{h}", name=f"attT{h}")
                nc.vector.tensor_copy(attT[0:64, :], ps_t[:])
                nc.vector.tensor_copy(attT[64:128, :], ps_t[:])
                attTs.append(attT)

            for p in range(2):
                yst = ypool.tile([128, HW], BF16, tag="yst")
                for vb in range(4):
                    vt = vpool.tile([128, 4096], BF16, tag="vt")
                    src_v = vs.ap()[2 * p:2 * p + 2, :, vb * 4096:(vb + 1) * 4096]
                    nc.scalar.dma_start(vt[:], src_v.rearrange("c t p -> (c t) p"))
                    for ci in range(2):
                        attT = attTs[p]
                        for j in range(8):
                            ps_y = psypool.tile([T, 512], F32, tag="psy")
                            nc.tensor.matmul(ps_y[:], attT[ci * 64:ci * 64 + 64, :],
                                             vt[ci * 64:ci * 64 + 64, j * 512:(j + 1) * 512],
                                             start=True, stop=True)
                            col = vb * 4096 + j * 512
                            nc.vector.tensor_copy(
                                yst[ci * 64:ci * 64 + 64, col:col + 512], ps_y[:])
                nc.sync.dma_start(ys.ap()[p], yst[:])
    nc.compile()
    return nc


def _build_l3():
    nc = bacc.Bacc("TRN2", target_bir_lowering=False, debug=False)
    planes = nc.dram_tensor("planes", (IPC, NPL, HP * WP), BF16, kind="ExternalInput")
    lhsT0 = nc.dram_tensor("lhsT0", (97, 16), BF16, kind="ExternalInput")
    lhsT1 = nc.dram_tensor("lhsT1", (48, 16), BF16, kind="ExternalInput")
    out = nc.dram_tensor("out", (IPC, 128, 4096), F32, kind="ExternalOutput")

    with tile.TileContext(nc) as tc:
        with tc.tile_pool(name="w", bufs=1) as wpool, \
             tc.tile_pool(name="pl", bufs=3) as plpool, \
             tc.tile_pool(name="st", bufs=3) as stpool, \
             tc.tile_pool(name="ps", bufs=4, space="PSUM") as pspool:
            w0 = wpool.tile([97, 16], BF16, tag="w0")
            w1 = wpool.tile([48, 16], BF16, tag="w1")
            nc.sync.dma_start(w0[:], lhsT0.ap())
            nc.sync.dma_start(w1[:], lhsT1.ap())

            def rhs_view(pt, nrows, blk, ky):
                base = (blk * 4 + ky) * WP
                return pt[0:nrows, base:base + 4 * WP].rearrange(
                    "p (h w) -> p h w", w=WP)[:, :, 0:W]

            for img in range(IPC):
                pt = plpool.tile([NPL, HP * WP], BF16)
                nc.scalar.dma_start(pt[:], planes.ap()[img])
                stage = stpool.tile([128, 4096], F32)
                for q4 in range(8):
                    ps = pspool.tile([128, 1024], F32)
                    for sub in range(2):
                        for half in range(2):
                            blk = q4 * 4 + sub * 2 + half
                            po = half * 64
                            psv = ps[po:po + 16, sub * 512:sub * 512 + 512]
                            nc.tensor.matmul(psv, w0[:], rhs_view(pt, 97, blk, 0),
                                             start=True, stop=False,
                                             tile_position=(0, po))
                            nc.tensor.matmul(psv, w1[:], rhs_view(pt, 48, blk, 2),
                                             start=False, stop=True,
                                             tile_position=(0, po))
                    for half in range(2):
                        slot = 2 * (q4 % 2) + half
                        nc.vector.tensor_copy(
                            stage[32 * slot:32 * slot + 16,
                                  (q4 // 2) * 1024:(q4 // 2) * 1024 + 1024],
                            ps[half * 64:half * 64 + 16, :])
                nc.sync.dma_start(out.ap()[img], stage[:])
    nc.compile()
    return nc


def _get(name):
    if name not in _BUILD_CACHE:
        _BUILD_CACHE[name] = {"l1": _build_l1, "l2": _build_l2, "l3": _build_l3}[name]()
    return _BUILD_CACHE[name]


# ---------------- host-side packing ----------------

def _build_planes(imgs_chw):
    """imgs_chw: [N, 16, H, W] float32-like -> [N, 98, HP*WP] bf16."""
    N = imgs_chw.shape[0]
    xpad = np.zeros((N, C, HP, WP), np.float32)
    xpad[:, :, 1:H + 1, 1:W + 1] = imgs_chw.astype(np.float32)
    flat = xpad.reshape(N, C, HP * WP)
    p = np.zeros((N, NPL, HP * WP), np.float32)
    p[:, 0:16] = flat
    p[:, 16:32, :-1] = flat[:, :, 1:]
    p[:, 32:48, :-2] = flat[:, :, 2:]
    p[:, 48] = 1.0
    p[:, 49:97, :-WP] = p[:, 0:48, WP:]
    return p.astype(NPBF16)


def _build_lhsT(ws, bs):
    """ws: list of [O,C,3,3]; bs: list of [O] -> lhsT0 [97, 16*len], lhsT1 [48, 16*len]."""
    n = len(ws)
    m = np.zeros((3, 49, 16 * n), np.float32)
    for j, (w, b) in enumerate(zip(ws, bs)):
        for ky in range(3):
            for kx in range(3):
                m[ky, kx * 16:(kx + 1) * 16, j * 16:(j + 1) * 16] = w[:, :, ky, kx].T
        m[1, 48, j * 16:(j + 1) * 16] = b
    l0 = np.zeros((97, 16 * n), np.float32)
    l0[0:48] = m[0][0:48]
    l0[48] = m[1][48]
    l0[49:97] = m[1][0:48]
    return l0.astype(NPBF16), m[2][0:48].astype(NPBF16)


def _unpack_qkv(qkv_out):
    """[N,128,8192] bf16 -> q,k,v each [N,16,HW].

    blk = q4*4 + sub*2 + half lives at stage rows half*64(+48), col q4*1024+sub*512."""
    N = qkv_out.shape[0]
    s = qkv_out.reshape(N, 128, 8, 2, 512)       # [N, p, q4, sub, 512]
    out = np.empty((N, 48, 8, 2, 2, 512), qkv_out.dtype)  # [N, c, q4, sub, half, 512]
    out[..., 0, :] = s[:, 0:48]
    out[..., 1, :] = s[:, 64:112]
    out = out.reshape(N, 48, HW)
    return out[:, 0:16], out[:, 16:32], out[:, 32:48]


def _unpack_l3(o):
    """[N,128,4096] f32 -> [N,16,HW].

    blk = q4*4 + sub*2 + half lives at stage row 32*(2*(q4%2)+half)+c,
    col (q4//2)*1024 + sub*512."""
    N = o.shape[0]
    s = o.reshape(N, 4, 32, 4, 2, 512)   # [N, slot, 32row, q4//2, sub, 512]
    s = s[:, :, 0:16]                    # [N, slot, c, q4h, sub, 512]
    out = np.empty((N, 16, 32, 512), o.dtype)
    for slot in range(4):
        half, q4lo = slot % 2, slot // 2
        for q4h in range(4):
            q4 = 2 * q4h + q4lo
            for sub in range(2):
                blk = q4 * 4 + sub * 2 + half
                out[:, :, blk] = s[:, slot, :, q4h, sub]
    return np.ascontiguousarray(out).reshape(N, 16, HW)


# ---------------- top level ----------------

def kernel(x, wq, bq, wk, bk, wv, bv, wo, bo):
    x, wq, bq, wk, bk, wv, bv, wo, bo = (
        np.asarray(a, np.float32) for a in (x, wq, bq, wk, bk, wv, bv, wo, bo))
    ximg = x.reshape(IMGS, C, H, W)
    cores = list(range(NCORES))

    # ---- L1: q/k/v convs, image-sharded
    l0, l1 = _build_lhsT([wq, wk, wv], [bq, bk, bv])
    in_maps = [{"planes": _build_planes(ximg[c * IPC:(c + 1) * IPC]),
                "lhsT0": l0, "lhsT1": l1} for c in cores]
    res1 = bass_utils.run_bass_kernel_spmd(_get("l1"), in_maps, core_ids=cores)

    # assemble channel-major [B, 16, T, HW] bf16
    q_all = np.empty((B, 16, T, HW), NPBF16)
    k_all = np.empty_like(q_all)
    v_all = np.empty_like(q_all)
    for c in cores:
        q, k, v = _unpack_qkv(res1.results[c]["qkv_out"])
        b0 = (c * IPC) // T
        t0 = (c * IPC) % T
        q_all[b0, :, t0:t0 + IPC] = q.transpose(1, 0, 2)
        k_all[b0, :, t0:t0 + IPC] = k.transpose(1, 0, 2)
        v_all[b0, :, t0:t0 + IPC] = v.transpose(1, 0, 2)

    # ---- L2: attention, head-sharded (2 heads = 4 channels per core)
    mask = np.triu(np.full((T, T), -30000.0, np.float32), 1)
    ident = np.eye(T, dtype=NPBF16)
    in_maps = []
    for c in cores:
        b, g = c // 4, c % 4
        sl = slice(4 * g, 4 * g + 4)
        qks = np.concatenate([q_all[b, sl], k_all[b, sl]], axis=0)
        in_maps.append({"qks": np.ascontiguousarray(qks),
                        "vs": np.ascontiguousarray(v_all[b, sl]),
                        "mask": mask, "ident": ident})
    res2 = bass_utils.run_bass_kernel_spmd(_get("l2"), in_maps, core_ids=cores)

    y_all = np.empty((B, 16, T, HW), NPBF16)
    for c in cores:
        b, g = c // 4, c % 4
        ys = res2.results[c]["ys"]
        for p in range(2):
            y_all[b, 4 * g + 2 * p] = ys[p, 0:64]
            y_all[b, 4 * g + 2 * p + 1] = ys[p, 64:128]

    # ---- L3: output conv, image-sharded
    yimg = y_all.astype(np.float32).transpose(0, 2, 1, 3).reshape(IMGS, 16, H, W)
    l0o, l1o = _build_lhsT([wo], [bo])
    in_maps = [{"planes": _build_planes(yimg[c * IPC:(c + 1) * IPC]),
                "lhsT0": l0o, "lhsT1": l1o} for c in cores]
    res3 = bass_utils.run_bass_kernel_spmd(_get("l3"), in_maps, core_ids=cores)

    out = np.concatenate([_unpack_l3(res3.results[c]["out"]) for c in cores])
    return np.ascontiguousarray(out.reshape(B, T, O, H, W))


# revision 7
# speedup vs baseline: 1.5361x; 1.1557x over previous
"""Trainium2 Bass kernel for nn_CNNT_enhanced_denoising_runtime_53704271069472.

Computes, distributed across 8 NeuronCores:
    q/k/v = conv3x3(x, w?, b?)          (image-sharded: B*T=128 imgs, 16/core)
    att   = causal-softmax(q @ k^T / sqrt(D)) per (batch, head)
    y     = att @ v                      (head-sharded: 16 (b,head) pairs, 2/core)
    out   = conv3x3(y, wo, bo)           (image-sharded)

Three SPMD launches with host-side resharding between them. Convs are done as
matmuls over K = (3 kx-taps x 16 ch [+ ones bias row]) x 2 ky-rows = 97(+48)
against kx-pre-shifted zero-padded image planes built on the host; the 3x3
kernel's third ky row is a second accumulating matmul with an AP row offset.
Compute dtype bf16 (fp32 PSUM accumulation).
"""
import sys
import numpy as np

sys.path.insert(0, "/opt/trn_rl_repo")

import ml_dtypes  # noqa: E402
import concourse.bacc as bacc  # noqa: E402
import concourse.tile as tile  # noqa: E402
import concourse.bass as bass  # noqa: E402
from concourse import mybir, bass_utils  # noqa: E402

BF16 = mybir.dt.bfloat16
F32 = mybir.dt.float32
NPBF16 = ml_dtypes.bfloat16

B, T, C, H, W, O = 2, 64, 16, 128, 128, 16
HP, WP = H + 2, W + 2
HW = H * W
NH, HC = 8, 2
D = HC * HW
SCALE = float(1.0 / np.sqrt(np.float32(D)))
NCORES = 8
IMGS = B * T
IPC = IMGS // NCORES  # images per core
NPL = 98  # plane rows: 48 (ky0 kx-taps) + ones + 48 (ky1) + pad

_BUILD_CACHE = {}


# ---------------- device programs ----------------

def _build_l1():
    nc = bacc.Bacc("TRN2", target_bir_lowering=False, debug=False)
    planes = nc.dram_tensor("planes", (IPC, NPL, HP * WP), BF16, kind="ExternalInput")
    lhsT0 = nc.dram_tensor("lhsT0", (97, 48), BF16, kind="ExternalInput")
    lhsT1 = nc.dram_tensor("lhsT1", (48, 48), BF16, kind="ExternalInput")
    qkv = nc.dram_tensor("qkv_out", (IPC, 128, 8192), BF16, kind="ExternalOutput")

    with tile.TileContext(nc) as tc:
        with tc.tile_pool(name="w", bufs=1) as wpool, \
             tc.tile_pool(name="pl", bufs=3) as plpool, \
             tc.tile_pool(name="st", bufs=3) as stpool, \
             tc.tile_pool(name="ps", bufs=4, space="PSUM") as pspool:
            w0 = wpool.tile([97, 48], BF16, tag="w0")
            w1 = wpool.tile([48, 48], BF16, tag="w1")
            nc.sync.dma_start(w0[:], lhsT0.ap())
            nc.sync.dma_start(w1[:], lhsT1.ap())

            def rhs_view(pt, nrows, blk, ky):
                base = (blk * 4 + ky) * WP
                return pt[0:nrows, base:base + 4 * WP].rearrange(
                    "p (h w) -> p h w", w=WP)[:, :, 0:W]

            for img in range(IPC):
                pt = plpool.tile([NPL, HP * WP], BF16)
                nc.scalar.dma_start(pt[:], planes.ap()[img])
                stage = stpool.tile([128, 8192], BF16)
                for q4 in range(8):
                    ps = pspool.tile([128, 1024], F32)
                    for sub in range(2):
                        for half in range(2):
                            blk = q4 * 4 + sub * 2 + half
                            psv = ps[half * 64:half * 64 + 48,
                                     sub * 512:sub * 512 + 512]
                            nc.tensor.matmul(psv, w0[:], rhs_view(pt, 97, blk, 0),
                                             start=True, stop=False,
                                             tile_position=(0, half * 64))
                            nc.tensor.matmul(psv, w1[:], rhs_view(pt, 48, blk, 2),
                                             start=False, stop=True,
                                             tile_position=(0, half * 64))
                    nc.vector.tensor_copy(stage[:, q4 * 1024:(q4 + 1) * 1024], ps[:])
                nc.sync.dma_start(qkv.ap()[img], stage[:])
    nc.compile()
    return nc


def _build_l2():
    nc = bacc.Bacc("TRN2", target_bir_lowering=False, debug=False)
    qks = nc.dram_tensor("qks", (8, T, HW), BF16, kind="ExternalInput")
    vs = nc.dram_tensor("vs", (4, T, HW), BF16, kind="ExternalInput")
    mask = nc.dram_tensor("mask", (T, T), F32, kind="ExternalInput")
    ident = nc.dram_tensor("ident", (T, T), BF16, kind="ExternalInput")
    ys = nc.dram_tensor("ys", (2, 128, HW), BF16, kind="ExternalOutput")

    with tile.TileContext(nc) as tc:
        with tc.tile_pool(name="cst", bufs=1) as cst, \
             tc.tile_pool(name="qk", bufs=6) as qkpool, \
             tc.tile_pool(name="sm", bufs=2) as smpool, \
             tc.tile_pool(name="v", bufs=8) as vpool, \
             tc.tile_pool(name="yst", bufs=2) as ypool, \
             tc.tile_pool(name="pst", bufs=1, space="PSUM") as pstpool, \
             tc.tile_pool(name="psy", bufs=3, space="PSUM") as psypool, \
             tc.tile_pool(name="psl", bufs=1, space="PSUM") as pslpool:
            mask_t = cst.tile([T, T], F32, tag="mask")
            nc.sync.dma_start(mask_t[:], mask.ap())
            id_t = cst.tile([T, T], BF16, tag="ident")
            nc.sync.dma_start(id_t[:], ident.ap())

            lg_ps = [pslpool.tile([128, 128], F32, tag=f"lg{h}", name=f"lg{h}")
                     for h in range(2)]
            for blk in range(128):
                qkt = qkpool.tile([128, 512], BF16, tag="qkT")
                src = qks.ap()[:, :, blk * 128:(blk + 1) * 128].rearrange("c t p -> (c t) p")
                eng = nc.scalar if blk % 2 else nc.sync
                eng.dma_start_transpose(qkt[:], src)
                for h in range(2):
                    nc.tensor.matmul(lg_ps[h][:],
                                     qkt[:, h * 128:(h + 1) * 128],
                                     qkt[:, 256 + h * 128:256 + (h + 1) * 128],
                                     start=(blk == 0), stop=(blk == 127))

            attTs = []
            for h in range(2):
                lg = smpool.tile([T, T], F32, tag="lg")
                nc.vector.tensor_copy(lg[:], lg_ps[h][0:64, 0:64])
                nc.vector.tensor_add(lg[:], lg[:], lg_ps[h][64:128, 64:128])
                nc.vector.tensor_scalar(lg[:], lg[:], SCALE, None,
                                        op0=mybir.AluOpType.mult)
                nc.vector.tensor_add(lg[:], lg[:], mask_t[:])
                mx = smpool.tile([T, 1], F32, tag="mx")
                nc.vector.reduce_max(mx[:], lg[:], axis=mybir.AxisListType.X, negate=True)
                e = smpool.tile([T, T], F32, tag="e")
                sm_acc = smpool.tile([T, 1], F32, tag="smacc")
                nc.scalar.activation(e[:], lg[:], mybir.ActivationFunctionType.Exp,
                                     bias=mx[:], scale=1.0, accum_out=sm_acc[:])
                rc = smpool.tile([T, 1], F32, tag="rc")
                nc.vector.reciprocal(rc[:], sm_acc[:])
                att = smpool.tile([T, T], BF16, tag="att")
                nc.vector.tensor_scalar(att[:], e[:], rc[:], None,
                                        op0=mybir.AluOpType.mult)
                ps_t = pstpool.tile([T, T], BF16, tag="pst")
                nc.tensor.transpose(ps_t[:], att[:], id_t[:])
                attT = smpool.tile([128, T], BF16, tag=f"attT{h}", name=f"attT{h}")
                nc.vector.tensor_copy(attT[0:64, :], ps_t[:])
                nc.vector.tensor_copy(attT[64:128, :], ps_t[:])
                attTs.append(attT)

            for p in range(2):
                yst = ypool.tile([128, HW], BF16, tag="yst")
                for vb in range(4):
                    vt = vpool.tile([128, 4096], BF16, tag="vt")
                    src_v = vs.ap()[2 * p:2 * p + 2, :, vb * 4096:(vb + 1) * 4096]
                    nc.scalar.dma_start(vt[:], src_v.rearrange("c t p -> (c t) p"))
                    for ci in range(2):
                        attT = attTs[p]
                        for j in range(8):
                            ps_y = psypool.tile([T, 512], F32, tag="psy")
                            nc.tensor.matmul(ps_y[:], attT[ci * 64:ci * 64 + 64, :],
                                             vt[ci * 64:ci * 64 + 64, j * 512:(j + 1) * 512],
                                             start=True, stop=True)
                            col = vb * 4096 + j * 512
                            nc.vector.tensor_copy(
                                yst[ci * 64:ci * 64 + 64, col:col + 512], ps_y[:])
                nc.sync.dma_start(ys.ap()[p], yst[:])
    nc.compile()
    return nc


def _build_l3():
    nc = bacc.Bacc("TRN2", target_bir_lowering=False, debug=False)
    planes = nc.dram_tensor("planes", (IPC, NPL, HP * WP), BF16, kind="ExternalInput")
    lhsT0 = nc.dram_tensor("lhsT0", (97, 16), BF16, kind="ExternalInput")
    lhsT1 = nc.dram_tensor("lhsT1", (48, 16), BF16, kind="ExternalInput")
    out = nc.dram_tensor("out", (IPC, 128, 4096), F32, kind="ExternalOutput")

    with tile.TileContext(nc) as tc:
        with tc.tile_pool(name="w", bufs=1) as wpool, \
             tc.tile_pool(name="pl", bufs=3) as plpool, \
             tc.tile_pool(name="st", bufs=3) as stpool, \
             tc.tile_pool(name="ps", bufs=4, space="PSUM") as pspool:
            w0 = wpool.tile([97, 16], BF16, tag="w0")
            w1 = wpool.tile([48, 16], BF16, tag="w1")
            nc.sync.dma_start(w0[:], lhsT0.ap())
            nc.sync.dma_start(w1[:], lhsT1.ap())

            def rhs_view(pt, nrows, blk, ky):
                base = (blk * 4 + ky) * WP
                return pt[0:nrows, base:base + 4 * WP].rearrange(
                    "p (h w) -> p h w", w=WP)[:, :, 0:W]

            for img in range(IPC):
                pt = plpool.tile([NPL, HP * WP], BF16)
                nc.scalar.dma_start(pt[:], planes.ap()[img])
                stage = stpool.tile([128, 4096], F32)
                for q4 in range(8):
                    ps = pspool.tile([128, 1024], F32)
                    for sub in range(2):
                        for half in range(2):
                            blk = q4 * 4 + sub * 2 + half
                            po = half * 64
                            psv = ps[po:po + 16, sub * 512:sub * 512 + 512]
                            nc.tensor.matmul(psv, w0[:], rhs_view(pt, 97, blk, 0),
                                             start=True, stop=False,
                                             tile_position=(0, po))
                            nc.tensor.matmul(psv, w1[:], rhs_view(pt, 48, blk, 2),
                                             start=False, stop=True,
                                             tile_position=(0, po))
                    for half in range(2):
                        slot = 2 * (q4 % 2) + half
                        dst = stage[32 * slot:32 * slot + 16,
                                    (q4 // 2) * 1024:(q4 // 2) * 1024 + 1024]
                        srcv = ps[half * 64:half * 64 + 16, :]
                        if half == 1:
                            nc.scalar.activation(dst, srcv,
                                                 mybir.ActivationFunctionType.Copy)
                        else:
                            nc.vector.tensor_copy(dst, srcv)
                nc.sync.dma_start(out.ap()[img], stage[:])
    nc.compile()
    return nc


def _get(name):
    if name not in _BUILD_CACHE:
        _BUILD_CACHE[name] = {"l1": _build_l1, "l2": _build_l2, "l3": _build_l3}[name]()
    return _BUILD_CACHE[name]


# ---------------- host-side packing ----------------

def _build_planes(imgs_chw):
    """imgs_chw: [N, 16, H, W] float32-like -> [N, 98, HP*WP] bf16."""
    N = imgs_chw.shape[0]
    xpad = np.zeros((N, C, HP, WP), np.float32)
    xpad[:, :, 1:H + 1, 1:W + 1] = imgs_chw.astype(np.float32)
    flat = xpad.reshape(N, C, HP * WP)
    p = np.zeros((N, NPL, HP * WP), np.float32)
    p[:, 0:16] = flat
    p[:, 16:32, :-1] = flat[:, :, 1:]
    p[:, 32:48, :-2] = flat[:, :, 2:]
    p[:, 48] = 1.0
    p[:, 49:97, :-WP] = p[:, 0:48, WP:]
    return p.astype(NPBF16)


def _build_lhsT(ws, bs):
    """ws: list of [O,C,3,3]; bs: list of [O] -> lhsT0 [97, 16*len], lhsT1 [48, 16*len]."""
    n = len(ws)
    m = np.zeros((3, 49, 16 * n), np.float32)
    for j, (w, b) in enumerate(zip(ws, bs)):
        for ky in range(3):
            for kx in range(3):
                m[ky, kx * 16:(kx + 1) * 16, j * 16:(j + 1) * 16] = w[:, :, ky, kx].T
        m[1, 48, j * 16:(j + 1) * 16] = b
    l0 = np.zeros((97, 16 * n), np.float32)
    l0[0:48] = m[0][0:48]
    l0[48] = m[1][48]
    l0[49:97] = m[1][0:48]
    return l0.astype(NPBF16), m[2][0:48].astype(NPBF16)


def _unpack_qkv(qkv_out):
    """[N,128,8192] bf16 -> q,k,v each [N,16,HW].

    blk = q4*4 + sub*2 + half lives at stage rows half*64(+48), col q4*1024+sub*512."""
    N = qkv_out.shape[0]
    s = qkv_out.reshape(N, 128, 8, 2, 512)       # [N, p, q4, sub, 512]
    out = np.empty((N, 48, 8, 2, 2, 512), qkv_out.dtype)  # [N, c, q4, sub, half, 512]
    out[..., 0, :] = s[:, 0:48]
    out[..., 1, :] = s[:, 64:112]
    out = out.reshape(N, 48, HW)
    return out[:, 0:16], out[:, 16:32], out[:, 32:48]


def _unpack_l3(o):
    """[N,128,4096] f32 -> [N,16,HW].

    blk = q4*4 + sub*2 + half lives at stage row 32*(2*(q4%2)+half)+c,
    col (q4//2)*1024 + sub*512."""
    N = o.shape[0]
    s = o.reshape(N, 4, 32, 4, 2, 512)   # [N, slot, 32row, q4//2, sub, 512]
    s = s[:, :, 0:16]                    # [N, slot, c, q4h, sub, 512]
    out = np.empty((N, 16, 32, 512), o.dtype)
    for slot in range(4):
        half, q4lo = slot % 2, slot // 2
        for q4h in range(4):
            q4 = 2 * q4h + q4lo
            for sub in range(2):
                blk = q4 * 4 + sub * 2 + half
                out[:, :, blk] = s[:, slot, :, q4h, sub]
    return np.ascontiguousarray(out).reshape(N, 16, HW)


# ---------------- top level ----------------

def kernel(x, wq, bq, wk, bk, wv, bv, wo, bo):
    x, wq, bq, wk, bk, wv, bv, wo, bo = (
        np.asarray(a, np.float32) for a in (x, wq, bq, wk, bk, wv, bv, wo, bo))
    ximg = x.reshape(IMGS, C, H, W)
    cores = list(range(NCORES))

    # ---- L1: q/k/v convs, image-sharded
    l0, l1 = _build_lhsT([wq, wk, wv], [bq, bk, bv])
    in_maps = [{"planes": _build_planes(ximg[c * IPC:(c + 1) * IPC]),
                "lhsT0": l0, "lhsT1": l1} for c in cores]
    res1 = bass_utils.run_bass_kernel_spmd(_get("l1"), in_maps, core_ids=cores)

    # assemble channel-major [B, 16, T, HW] bf16
    q_all = np.empty((B, 16, T, HW), NPBF16)
    k_all = np.empty_like(q_all)
    v_all = np.empty_like(q_all)
    for c in cores:
        q, k, v = _unpack_qkv(res1.results[c]["qkv_out"])
        b0 = (c * IPC) // T
        t0 = (c * IPC) % T
        q_all[b0, :, t0:t0 + IPC] = q.transpose(1, 0, 2)
        k_all[b0, :, t0:t0 + IPC] = k.transpose(1, 0, 2)
        v_all[b0, :, t0:t0 + IPC] = v.transpose(1, 0, 2)

    # ---- L2: attention, head-sharded (2 heads = 4 channels per core)
    mask = np.triu(np.full((T, T), -30000.0, np.float32), 1)
    ident = np.eye(T, dtype=NPBF16)
    in_maps = []
    for c in cores:
        b, g = c // 4, c % 4
        sl = slice(4 * g, 4 * g + 4)
        qks = np.concatenate([q_all[b, sl], k_all[b, sl]], axis=0)
        in_maps.append({"qks": np.ascontiguousarray(qks),
                        "vs": np.ascontiguousarray(v_all[b, sl]),
                        "mask": mask, "ident": ident})
    res2 = bass_utils.run_bass_kernel_spmd(_get("l2"), in_maps, core_ids=cores)

    y_all = np.empty((B, 16, T, HW), NPBF16)
    for c in cores:
        b, g = c // 4, c % 4
        ys = res2.results[c]["ys"]
        for p in range(2):
            y_all[b, 4 * g + 2 * p] = ys[p, 0:64]
            y_all[b, 4 * g + 2 * p + 1] = ys[p, 64:128]

    # ---- L3: output conv, image-sharded
    yimg = y_all.astype(np.float32).transpose(0, 2, 1, 3).reshape(IMGS, 16, H, W)
    l0o, l1o = _build_lhsT([wo], [bo])
    in_maps = [{"planes": _build_planes(yimg[c * IPC:(c + 1) * IPC]),
                "lhsT0": l0o, "lhsT1": l1o} for c in cores]
    res3 = bass_utils.run_bass_kernel_spmd(_get("l3"), in_maps, core_ids=cores)

    out = np.concatenate([_unpack_l3(res3.results[c]["out"]) for c in cores])
    return np.ascontiguousarray(out.reshape(B, T, O, H, W))
